# revision 1
# baseline (speedup 1.0000x reference)
"""DiT forward kernel for 8x Trainium2 NeuronCores (Bass/Tile).

Sharding: data-parallel over batch. Core b computes batch element b end to
end; weights are replicated (host-cast to bf16) across cores.

v2: software-pipelined block structure. All work that does not depend on
the residual stream (weight DMAs, the adaLN row-vector chain, and the
pe-mask chain pe1/pe2/groupnorm/sigmoid) is emitted one block ahead
("prep"), so the scheduler can overlap it with the xc-dependent critical
path ("main") of the previous block. Weight DMAs ride the otherwise-idle
GpSimd SWDGE queue. GELU uses the sigmoid approximation built from Exp so
the whole steady-state loop stays inside the natural_log_exp activation
table set (no per-block table reloads).

Layouts per core:
  - residual stream xc: tok-major fp32 SBUF [128, 2, 512]
      (partition = token % 128, chunk = token // 128, free = channel)
  - matmul activations: chan-major bf16 [128, S, 256]
  - weights W^T staged [128, S, N] bf16 (partition = in-chan % 128,
      sub = in-chan // 128, free = out-chan).

Attention is computed key-major: sT = k q^T, the pe-mask is built directly
transposed, softmax runs un-normalized (exp without max subtraction -- the
scores here are O(1)), and the normalizer Z comes from a ones-column
appended to v; the divide is fused into the per-head output copy.
"""

import math
import os
import sys

import numpy as np

try:
    import concourse.bass as bass
except Exception:
    sys.path.insert(0, "/opt/trn_rl_repo")
    import concourse.bass as bass

import ml_dtypes
from contextlib import ExitStack

import concourse.mybir as mybir
import concourse.tile as tile
from concourse import bacc
from concourse.bass_utils import run_bass_kernel_spmd
import concourse.bacc as bacc_mod
import concourse.hw_specs as hw_specs

BF16 = mybir.dt.bfloat16
F32 = mybir.dt.float32
AF = mybir.ActivationFunctionType
ALU = mybir.AluOpType
AX = mybir.AxisListType
ts = bass.ts

P = 128
B, TOK, HID, LAT, OUT_C, HEADS = 8, 256, 128, 512, 256, 8
DEPTH = int(os.environ.get("DIT_DEPTH", "36"))
MLP_H = 2048
HD = LAT // HEADS  # 64
FREQ = 256
TC = TOK // P      # 2 token chunks
LS = LAT // P      # 4 channel subtiles
MS = MLP_H // P    # 16
PI = math.pi
TWO_PI = 2.0 * math.pi


# ---------------------------------------------------------------------------
# device program helpers
# ---------------------------------------------------------------------------


def _range_reduce(nc, pool, x_ap, shape, tag, tags=None):
    """x >= 0 -> x mod 2pi folded into [-pi, pi), via int cast round/trunc."""
    tags = tags or (f"rr_t_{tag}", f"rr_i_{tag}", f"rr_r_{tag}")
    t = pool.tile(shape, F32, tag=tags[0])
    nc.vector.tensor_scalar(t[:], x_ap, 1.0 / TWO_PI, None, ALU.mult)
    ti = pool.tile(shape, mybir.dt.int32, tag=tags[1])
    nc.vector.tensor_copy(ti[:], t[:])
    nc.vector.tensor_copy(t[:], ti[:])
    red = pool.tile(shape, F32, tag=tags[2])
    nc.vector.scalar_tensor_tensor(red[:], t[:], -TWO_PI, x_ap, ALU.mult,
                                   ALU.add)
    nc.vector.tensor_scalar(t[:], red[:], PI, None, ALU.is_ge)
    nc.vector.scalar_tensor_tensor(red[:], t[:], -TWO_PI, red[:], ALU.mult,
                                   ALU.add)
    return red


def _ln_stats(nc, small, src_ap, eps_ap):
    """Free-dim LayerNorm stats of [128, N] fp32 -> (rstd, -mean*rstd)."""
    bnst = small.tile([P, 6], F32, tag="bnst")
    nc.vector.bn_stats(bnst[:], src_ap)
    mv = small.tile([P, 2], F32, tag="mv")
    nc.vector.bn_aggr(mv[:], bnst[:])
    sd = small.tile([P, 1], F32, tag="sd")
    nc.scalar.activation(sd[:], mv[:, 1:2], AF.Ln, bias=eps_ap)
    r = small.tile([P, 1], F32, tag="rstd")
    nc.scalar.activation(r[:], sd[:], AF.Exp, scale=-0.5)
    mb = small.tile([P, 1], F32, tag="mb")
    nc.vector.tensor_scalar(mb[:], mv[:, 0:1], r[:], -1.0, ALU.mult, ALU.mult)
    return r, mb


def _ln_modulate_transpose(nc, small, xn_pool, psum, ident, xc, sc_ap, sh_ap,
                           eps_ap, hhT, tag):
    """LayerNorm(tok-major fp32 xc) -> chan-major, *(1+sc)+sh -> bf16 hhT."""
    xn = xn_pool.tile([P, TC, LAT], BF16, tag=f"xn_{tag}")
    for c in range(TC):
        r, mb = _ln_stats(nc, small, xc[:, c, :], eps_ap)
        nc.scalar.activation(xn[:, c, :], xc[:, c, :], AF.Identity,
                             bias=mb[:], scale=r[:])
    for s in range(LS):
        # both token chunks of one channel subtile share the modulate scale,
        # so transpose them into one psum tile and evacuate with one op
        pst = psum.tile([P, TC, P], BF16, tag="ps")
        for c in range(TC):
            nc.tensor.transpose(pst[:, c, :], xn[:, c, ts(s, P)], ident[:])
        nc.vector.tensor_scalar(
            hhT[:, s, :], pst[:],
            sc_ap[:, s:s + 1], sh_ap[:, s:s + 1], ALU.mult, ALU.add)


def _filtered_act_tables(arch):
    """Activation-table view for the table-load inserter: expose Ln/Exp only
    in the combined natural_log_exp set (and Sin/Silu only in silu_and_others)
    so the greedy per-function chooser doesn't thrash between the ln-only and
    exp-only sets on every LayerNorm/softmax pair. Set ids/order unchanged --
    every inserted load still names a set that really contains the function.
    """
    AFt = mybir.ActivationFunctionType
    out = {}
    for name, fns in hw_specs.get_activation_tables(arch).items():
        fns = set(fns)
        if name != "natural_log_exp_and_others":
            fns.discard(AFt.Exp)
            fns.discard(AFt.Ln)
        if name != "silu_and_others":
            fns.discard(AFt.Sin)
            fns.discard(AFt.Silu)
        out[name] = fns
    return out


def build_program(depth=DEPTH):
    nc = bacc.Bacc("TRN2", target_bir_lowering=False, debug=False,
                   num_devices=8)

    def din(name, shape, dt):
        return nc.dram_tensor(name, list(shape), dt,
                              kind="ExternalInput").ap()

    x_t = din("x_t", [P, TC, HID], F32)
    coords_t = din("coords_t", [P, TC, 3], F32)
    ident_d = din("ident", [P, P], BF16)
    div_bc_d = din("div_bc", [P, TC, 256], F32)
    cvec_d = din("cvec", [P, 8], F32)
    sel2T_d = din("sel2T", [2, P], F32)
    projw_d = din("projw", [P, 1, LAT], BF16)
    te1_d = din("te1", [P, 2, LAT], BF16)
    te2_d = din("te2", [P, LS, LAT], BF16)
    we_d = din("we_w", [depth, P, LS, 8 * LAT], BF16)  # [ada(6L)|pe1|pe2]
    qpv_d = din("qpv_w", [depth, P, LS, 4 * LAT], BF16)
    f1_d = din("f1_w", [depth, P, LS, MLP_H], BF16)
    f2_d = din("f2_w", [depth, P, MS, LAT], BF16)
    finada_d = din("finada", [P, LS, 2 * LAT], BF16)
    finw_d = din("finw", [P, LS, OUT_C], BF16)

    out_t = nc.dram_tensor("out_t", [P, TC, OUT_C], F32,
                           kind="ExternalOutput").ap()

    with tile.TileContext(nc) as tc:
        with ExitStack() as ctx:
            _emit(ctx, tc, nc, depth, x_t, coords_t, ident_d,
                  div_bc_d, cvec_d, sel2T_d, projw_d, te1_d, te2_d,
                  we_d, qpv_d, f1_d, f2_d, finada_d, finw_d, out_t)
    orig_gat = bacc_mod.get_activation_tables
    bacc_mod.get_activation_tables = _filtered_act_tables
    try:
        nc.compile()
    finally:
        bacc_mod.get_activation_tables = orig_gat
    return nc


def _emit(ctx, tc, nc, depth, x_t, coords_t, ident_d, div_bc_d,
          cvec_d, sel2T_d, projw_d, te1_d, te2_d, we_d, qpv_d, f1_d, f2_d,
          finada_d, finw_d, out_t):
    def pool(name, bufs, space="SBUF"):
        return ctx.enter_context(
            tc.tile_pool(name=name, bufs=bufs, space=space))

    pers = pool("pers", 1)    # persistent state + consts
    once = pool("once", 1)    # prelude / final-layer temporaries
    wq = pool("wq", 1)        # per-block weights (single-buffered by chunk;
                              # use-timing gives natural double buffering)
    prep1 = pool("prep1", 1)  # prep-internal transients
    prep2 = pool("prep2", 2)  # prep outputs consumed by next block's main
    blk1 = pool("blk1", 1)    # within-block activations
    blk2x = pool("blk2x", 2)  # block-boundary activations (double-buffered)
    head3 = pool("head3", 2)  # per-head-pair attention temporaries
    small = pool("small", 3)  # tiny stat tiles
    psum = pool("psum", 4, space="PSUM")
    psp = pool("psp", 3, space="PSUM")
    psrow = pool("psrow", 1, space="PSUM")
    dram = pool("dram", 2, space="DRAM")

    # ---------------- persistent/consts ----------------
    ident = pers.tile([P, P], BF16, tag="ident")
    nc.sync.dma_start(ident[:], ident_d)
    cvec = pers.tile([P, 8], F32, tag="cvec")
    nc.sync.dma_start(cvec[:], cvec_d)
    eps5 = cvec[:, 1:2]
    eps6 = cvec[:, 2:3]
    sel2 = cvec[:, 5:7]

    sel2T = pers.tile([2, P], F32, tag="sel2T")
    nc.sync.dma_start(sel2T[:], sel2T_d)
    ones1 = pers.tile([1, P], BF16, tag="ones1")
    nc.vector.memset(ones1[:], 1.0)

    xc = pers.tile([P, TC, LAT], F32, tag="xc")
    v_aug = pers.tile([P, TC, HEADS, HD + 1], BF16, tag="v_aug")
    nc.vector.memset(v_aug[:, :, :, HD:HD + 1], 1.0)

    # ---------------- t embedding -> sT = silu(c) chan-major ----------------
    tf = small.tile([P, 1], F32, tag="tf")
    nc.vector.tensor_scalar(tf[:], cvec[:, 3:4], cvec[:, 4:5], None,
                            ALU.mult)
    embT = once.tile([P, 2, 1], BF16, tag="embT")
    for idx, off in ((0, PI / 2.0), (1, 0.0)):  # sub0=cos, sub1=sin
        xsh = small.tile([P, 1], F32, tag="tf_sh")
        nc.vector.tensor_scalar(xsh[:], tf[:], off, None, ALU.add)
        red = _range_reduce(nc, small, xsh[:], [P, 1], "emb")
        nc.scalar.activation(embT[:, idx, :], red[:], AF.Sin)
    te1 = once.tile([P, 2, LAT], BF16, tag="te1")
    nc.sync.dma_start(te1[:], te1_d)
    te2 = once.tile([P, LS, LAT], BF16, tag="te2")
    nc.sync.dma_start(te2[:], te2_d)
    ps_h1 = psrow.tile([1, LAT], F32, tag="ps_row")
    for s in range(2):
        nc.tensor.matmul(ps_h1[:], embT[:, s, :], te1[:, s, :],
                         start=(s == 0), stop=(s == 1))
    h1row = once.tile([1, 16 * P], BF16, tag="h1row")
    nc.vector.memset(h1row[:], 0.0)
    nc.scalar.activation(h1row[:, 0:LAT], ps_h1[:], AF.Silu)
    h1_dr = dram.tile([1, 16 * P], BF16, tag="h1_dr")
    nc.sync.dma_start(h1_dr[:], h1row[:])
    h1T = once.tile([P, 16], BF16, tag="h1T")
    nc.sync.dma_start_transpose(
        h1T[:], h1_dr[:].rearrange("o (r c) -> (o r) c", r=16, c=P))
    ps_c = psrow.tile([1, LAT], F32, tag="ps_row")
    for s in range(LS):
        nc.tensor.matmul(ps_c[:], h1T[:, s:s + 1], te2[:, s, :],
                         start=(s == 0), stop=(s == LS - 1))
    s_row = once.tile([1, 16 * P], BF16, tag="s_row")
    nc.vector.memset(s_row[:], 0.0)
    nc.scalar.activation(s_row[:, 0:LAT], ps_c[:], AF.Silu)
    s_dr = dram.tile([1, 16 * P], BF16, tag="s_dr")
    nc.sync.dma_start(s_dr[:], s_row[:])
    sT16 = pers.tile([P, 16], BF16, tag="sT")
    nc.sync.dma_start_transpose(
        sT16[:], s_dr[:].rearrange("o (r c) -> (o r) c", r=16, c=P))
    sT = sT16[:].rearrange("p (s o) -> p s o", o=1)

    # ---------------- positional encoding ----------------
    cds = once.tile([P, TC, 3], F32, tag="cds")
    nc.sync.dma_start(cds[:], coords_t)
    # prelude temporaries borrow block-loop tile slots (dead by block 0)
    div_bc = blk1.tile([P, TC, 256], F32, tag="mlpT")
    nc.sync.dma_start(div_bc[:], div_bc_d)
    enc = small.tile([P, TC, 1], F32, tag="enc")
    nc.vector.scalar_tensor_tensor(enc[:, :, 0], cds[:, :, 1], 100.0,
                                   cds[:, :, 2], ALU.mult, ALU.add)
    nc.vector.scalar_tensor_tensor(enc[:, :, 0], cds[:, :, 0], 10000.0,
                                   enc[:, :, 0], ALU.mult, ALU.add)
    ang = blk1.tile([P, TC, 256], F32, tag="qkT")
    nc.vector.tensor_tensor(ang[:], div_bc[:],
                            enc[:].to_broadcast((P, TC, 256)), ALU.mult)
    pe = pers.tile([P, TC, LAT], F32, tag="pe")
    pe4 = pe[:].rearrange("p c (j k) -> p c j k", j=256, k=2)
    for k, off in ((0, 0.0), (1, PI / 2.0)):  # even=sin, odd=cos
        xsh = blk1.tile([P, TC, 256], F32, tag="attn")
        nc.vector.tensor_scalar(xsh[:], ang[:], off, None, ALU.add)
        red = _range_reduce(nc, blk1, xsh[:], [P, TC, 256], "pe",
                            tags=("hh1T", "hh2T", "attnT"))
        nc.scalar.activation(pe4[:, :, :, k], red[:], AF.Sin)
    peb = blk1.tile([P, TC, LAT], BF16, tag="xn_l1")
    nc.vector.tensor_copy(peb[:], pe[:])
    peT = pers.tile([P, LS, TOK], BF16, tag="peT")
    for c in range(TC):
        for s in range(LS):
            pst = psum.tile([P, P], BF16, tag="ps")
            nc.tensor.transpose(pst[:], peb[:, c, ts(s, P)], ident[:])
            nc.vector.tensor_copy(peT[:, s, ts(c, P)], pst[:])

    # ---------------- input projection ----------------
    xin = once.tile([P, TC, HID], F32, tag="xin")
    nc.sync.dma_start(xin[:], x_t)
    xn0 = once.tile([P, TC, HID], BF16, tag="xn0")
    for c in range(TC):
        r, mb = _ln_stats(nc, small, xin[:, c, :], eps5)
        nc.scalar.activation(xn0[:, c, :], xin[:, c, :], AF.Identity,
                             bias=mb[:], scale=r[:])
    xn0T = once.tile([P, TOK], BF16, tag="xn0T")
    for c in range(TC):
        pst = psum.tile([P, P], BF16, tag="ps")
        nc.tensor.transpose(pst[:], xn0[:, c, :], ident[:])
        nc.vector.tensor_copy(xn0T[:, ts(c, P)], pst[:])
    projw = once.tile([P, 1, LAT], BF16, tag="projw")
    nc.sync.dma_start(projw[:], projw_d)
    for c in range(TC):
        ph = psum.tile([P, LAT], F32, tag="ps")
        nc.tensor.matmul(ph[:], xn0T[:, ts(c, P)], projw[:, 0, :],
                         start=True, stop=True)
        hsb = once.tile([P, LAT], F32, tag="h_sb")
        nc.scalar.activation(hsb[:], ph[:], AF.Copy)
        r, mb = _ln_stats(nc, small, hsb[:], eps5)
        hn = once.tile([P, LAT], F32, tag="hn")
        nc.scalar.activation(hn[:], hsb[:], AF.Identity, bias=mb[:],
                             scale=r[:])
        nc.vector.tensor_tensor(xc[:, c, :], hn[:], pe[:, c, :], ALU.add)

    # ---------------- block prep (runs one block ahead) ----------------
    inv_gn = 1.0 / (64.0 * 256.0)

    from contextlib import contextmanager

    @contextmanager
    def _prep_prio():
        """Emit prep work in a lower-priority band (+5000) so ready prep
        instructions never queue ahead of the current block's residual-chain
        ops on a shared engine; the band stays below the next block's base."""
        p0 = tc.cur_priority
        tc.cur_priority = p0 + 5000
        try:
            yield
        finally:
            tc.cur_priority = p0

    def pe_mats(w_sb, raw, st1_sum, st1_sq):
        """pe-branch matmuls -> raw (bf16) + per-(m, partition) sum/sumsq.
        Both the raw copy and the square run on ACT with accum_out, so the
        row sums come for free and DVE stays out of the chain."""
        for m in range(LS):
            pspt = psp.tile([P, TOK], F32, tag="psp")
            for s in range(LS):
                nc.tensor.matmul(pspt[:], w_sb[:, s, ts(m, P)],
                                 peT[:, s, :], start=(s == 0),
                                 stop=(s == LS - 1))
            nc.scalar.activation(raw[:, m, :], pspt[:], AF.Copy,
                                 accum_out=st1_sum[:, m:m + 1])
            sq = prep1.tile([P, TOK], BF16, tag="gn_sq")
            nc.scalar.activation(sq[:], pspt[:], AF.Square,
                                 accum_out=st1_sq[:, m:m + 1])

    def gn_scale(raw, abbc, br, dst_bf):
        for m in range(LS):
            nc.vector.tensor_scalar(
                dst_bf[:, m, :], raw[:, m, :],
                abbc[:, br, m, 0:1], abbc[:, br, m, 1:2], ALU.mult, ALU.add)

    def prep_a(d, st):
        """we DMA + ada chain -> adaT, gates."""
        we = wq.tile([P, LS, 8 * LAT], BF16, tag="we")
        nc.gpsimd.dma_start(we[:], we_d[d])
        st["we"] = we
        adaw = we[:, :, 0:6 * LAT]
        # ada row = silu(c) @ aw^T; host stages rows [sh_a|sc_a|sh_m|sc_m|g|g]
        arow = prep1.tile([1, 6 * LAT], BF16, tag="arow")
        for v6 in range(6):
            psa = psrow.tile([1, LAT], F32, tag="ps_row")
            for s in range(LS):
                nc.tensor.matmul(psa[:], sT[:, s, :],
                                 adaw[:, s, ts(v6, LAT)],
                                 start=(s == 0), stop=(s == LS - 1))
            nc.scalar.activation(arow[:, ts(v6, LAT)], psa[:], AF.Copy)
        # shift/scale rows to chan-major via one xbar DMA transpose
        ada_dr = dram.tile([1, 4 * LAT], BF16, tag="ada_dr")
        nc.sync.dma_start(ada_dr[:], arow[:, 0:4 * LAT])
        adaTb = prep2.tile([P, 16], BF16, tag="adaTb")
        nc.sync.dma_start_transpose(
            adaTb[:], ada_dr[:].rearrange("o (r c) -> (o r) c", r=16, c=P))
        adaT = prep2.tile([P, 4, LS], F32, tag="adaT")
        nc.vector.tensor_copy(
            adaT[:], adaTb[:].rearrange("p (v s) -> p v s", v=4))
        nc.vector.tensor_scalar(adaT[:, 1, :], adaT[:, 1, :], 1.0, None,
                                ALU.add)
        nc.vector.tensor_scalar(adaT[:, 3, :], adaT[:, 3, :], 1.0, None,
                                ALU.add)
        # gates broadcast across partitions = outer product with a ones row
        gg_bc = prep2.tile([P, 2, LAT], BF16, tag="gg_bc")
        for g in range(2):
            psg = psum.tile([P, LAT], F32, tag="ps")
            nc.tensor.matmul(psg[:], ones1[:],
                             arow[:, ts(4 + g, LAT)], start=True, stop=True)
            nc.scalar.activation(gg_bc[:, g, :], psg[:], AF.Copy)
        st["adaT"] = adaT
        st["ga_bc"], st["gm_bc"] = gg_bc[:, 0, :], gg_bc[:, 1, :]

    def prep_b(d, st):
        """pe1 branch matmuls."""
        p1raw = prep1.tile([P, LS, TOK], BF16, tag="p1raw")
        st1 = small.tile([P, 16], F32, tag="gn_st1")
        st["p1raw"], st["st1"] = p1raw, st1
        pe_mats(st["we"][:, :, 6 * LAT:7 * LAT], p1raw[:], st1[:, 0:4],
                st1[:, 8:12])

    def prep_c(d, st):
        """pe2 branch matmuls + fused groupnorm stats + scales."""
        p1raw, st1 = st["p1raw"], st["st1"]
        p2raw = prep1.tile([P, LS, TOK], BF16, tag="p2raw")
        pe_mats(st["we"][:, :, 7 * LAT:8 * LAT], p2raw[:], st1[:, 4:8],
                st1[:, 12:16])
        # one matmul folds sums over the two partition halves: gs[h, j]
        # with j = br*4+m for sums, 8+br*4+m for sums of squares
        gs = psp.tile([2, 16], F32, tag="psp")
        nc.tensor.matmul(gs[:], sel2, st1[:], start=True, stop=True)
        mu = small.tile([2, 8], F32, tag="gn_mu")
        nc.vector.tensor_scalar(mu[:], gs[:, 0:8], inv_gn, None, ALU.mult)
        m2 = small.tile([2, 8], F32, tag="gn_m2")
        nc.vector.tensor_scalar(m2[:], gs[:, 8:16], inv_gn, None, ALU.mult)
        msq = small.tile([2, 8], F32, tag="gn_msq")
        nc.scalar.activation(msq[:], mu[:], AF.Square)
        var = small.tile([2, 8], F32, tag="gn_var")
        nc.vector.tensor_tensor(var[:], m2[:], msq[:], ALU.subtract)
        sd = small.tile([2, 8], F32, tag="gn_sd")
        nc.scalar.activation(sd[:], var[:], AF.Ln, bias=eps5[0:2, :])
        ab = small.tile([2, 8, 2], F32, tag="gn_ab")
        nc.scalar.activation(ab[:, :, 0], sd[:], AF.Exp, scale=-0.5)
        nc.vector.scalar_tensor_tensor(ab[:, :, 1], mu[:], -1.0,
                                       ab[:, :, 0], ALU.mult, ALU.mult)
        # partition-half broadcast = matmul with the half-indicator columns
        psb = psp.tile([P, 16], F32, tag="psp")
        nc.tensor.matmul(psb[:], sel2T[:], ab[:, :, :], start=True,
                         stop=True)
        abbc = small.tile([P, 2, LS, 2], F32, tag="gn_abbc")
        nc.scalar.activation(
            abbc[:], psb[:].rearrange("p (br m k) -> p br m k", br=2, m=LS),
            AF.Copy)
        pe1n = prep1.tile([P, LS, TOK], BF16, tag="pe1n")
        gn_scale(p1raw[:], abbc, 0, pe1n[:])
        pe2n = prep1.tile([P, LS, TOK], BF16, tag="pe2n")
        gn_scale(p2raw[:], abbc, 1, pe2n[:])
        st["pe1n"], st["pe2n"] = pe1n, pe2n

    def prep_d(d, st):
        """mask matmuls + sigmoid -> maskT."""
        pe1n, pe2n = st["pe1n"], st["pe2n"]
        maskT = prep2.tile([P, TC, TOK], F32, tag="maskT")
        for mc in range(TC):
            psm = psp.tile([P, TOK], F32, tag="psp")
            for s in range(LS):
                nc.tensor.matmul(psm[:], pe2n[:, s, ts(mc, P)],
                                 pe1n[:, s, :], start=(s == 0),
                                 stop=(s == LS - 1))
            # sigmoid(x) = 1 / (1 + exp(-x)) -- stays in the exp table set
            en = prep1.tile([P, TOK], F32, tag="mask_en")
            nc.scalar.activation(en[:], psm[:], AF.Exp, scale=-1.0)
            nc.vector.tensor_scalar(en[:], en[:], 1.0, None, ALU.add)
            nc.vector.reciprocal(maskT[:, mc, :], en[:])
        st["maskT"] = maskT

    def prep_e(d, st):
        """remaining weight chunks for main(d); the residual gates fold into
        the proj/fc2 weights here (in-place, per output channel) so the main
        chain's residual update is a single add from PSUM."""
        qpv = wq.tile([P, LS, 4 * LAT], BF16, tag="qpv")
        nc.scalar.dma_start(qpv[:], qpv_d[d])
        f1w = wq.tile([P, LS, MLP_H], BF16, tag="f1w")
        nc.scalar.dma_start(f1w[:], f1_d[d])
        f2w = wq.tile([P, MS, LAT], BF16, tag="f2w")
        nc.gpsimd.dma_start(f2w[:], f2_d[d])
        pjw = qpv[:, :, 3 * LAT:4 * LAT]
        nc.vector.tensor_tensor(
            pjw, pjw,
            st["ga_bc"].rearrange("p (o n) -> p o n", o=1)
            .to_broadcast((P, LS, LAT)), ALU.mult)
        nc.vector.tensor_tensor(
            f2w[:], f2w[:],
            st["gm_bc"].rearrange("p (o n) -> p o n", o=1)
            .to_broadcast((P, MS, LAT)), ALU.mult)
        st["qpv"], st["f1w"], st["f2w"] = qpv, f1w, f2w

    def prep_block(d):
        st = {}
        prep_a(d, st)
        prep_b(d, st)
        prep_c(d, st)
        prep_d(d, st)
        prep_e(d, st)
        return st

    # ---------------- transformer blocks (main) ----------------

    def main_block(d, st, dn, nxt):
        """Emit main(d); prep slices for block d+1 are interleaved at fixed
        points so the scheduler spreads the independent work into the
        residual chain's stall windows."""
        adaT, maskT = st["adaT"], st["maskT"]
        ga_bc, gm_bc = st["ga_bc"], st["gm_bc"]
        qpv, f1w, f2w = st["qpv"], st["f1w"], st["f2w"]
        qkw = qpv[:, :, 0:2 * LAT]
        vw = qpv[:, :, 2 * LAT:3 * LAT]
        pjw = qpv[:, :, 3 * LAT:4 * LAT]

        # --- LN1 + modulate + transpose ---
        hh1T = blk2x.tile([P, LS, TOK], BF16, tag="hh1T")
        _ln_modulate_transpose(nc, small, blk1, psum, ident, xc[:],
                               adaT[:, 1, :], adaT[:, 0, :], eps6, hh1T[:],
                               "l1")
        if nxt is not None:
            with _prep_prio():
                prep_a(dn, nxt)

        # --- qk^T (chan-major) and v (tok-major, ones col appended) ---
        qkT = blk1.tile([P, 2 * LS, TOK], BF16, tag="qkT")
        for mp in range(LS):
            psq = psum.tile([P, 2, TOK], F32, tag="ps")
            for half in range(2):
                m = 2 * mp + half
                for s in range(LS):
                    nc.tensor.matmul(psq[:, half, :], qkw[:, s, ts(m, P)],
                                     hh1T[:, s, :], start=(s == 0),
                                     stop=(s == LS - 1))
            nc.scalar.activation(qkT[:, 2 * mp:2 * mp + 2, :], psq[:],
                                 AF.Copy)
        for c in range(TC):
            psv = psum.tile([P, LAT], F32, tag="ps")
            for s in range(LS):
                nc.tensor.matmul(psv[:], hh1T[:, s, ts(c, P)], vw[:, s, :],
                                 start=(s == 0), stop=(s == LS - 1))
            nc.scalar.activation(
                v_aug[:, c, :, 0:HD],
                psv[:].rearrange("p (h d) -> p h d", h=HEADS), AF.Copy)
        if nxt is not None:
            with _prep_prio():
                prep_b(dn, nxt)

        # --- attention, key-major scores ---
        attn = blk1.tile([P, TC, LAT], BF16, tag="attn")
        for hp in range(HEADS // 2):
            stf = head3.tile([P, 2, TC, TOK], BF16, tag="stf")
            for i in range(2):
                h = 2 * hp + i
                pbase = (h % 2) * HD
                qs = qkT[pbase:pbase + HD, h // 2, :]
                ks = qkT[pbase:pbase + HD, 4 + h // 2, :]
                pss = psum.tile([P, TC, TOK], F32, tag="ps")
                for kc in range(TC):
                    nc.tensor.matmul(pss[:, kc, :], ks[:, ts(kc, P)], qs,
                                     start=True, stop=True)
                nc.vector.tensor_tensor(stf[:, i], pss[:], maskT[:],
                                        ALU.mult)
            ptil = head3.tile([P, 2, TC, TOK], BF16, tag="ptil")
            nc.scalar.activation(ptil[:], stf[:], AF.Exp)
            for i in range(2):
                h = 2 * hp + i
                pso = psum.tile([P, TC, HD + 1], F32, tag="ps")
                for qc in range(TC):
                    for kc in range(TC):
                        nc.tensor.matmul(pso[:, qc, :],
                                         ptil[:, i, kc, ts(qc, P)],
                                         v_aug[:, kc, h, :],
                                         start=(kc == 0),
                                         stop=(kc == TC - 1))
                rz = small.tile([P, TC, 1], F32, tag="rz")
                nc.vector.reciprocal(rz[:], pso[:, :, HD:HD + 1])
                for qc in range(TC):
                    nc.vector.tensor_scalar(attn[:, qc, ts(h, HD)],
                                            pso[:, qc, 0:HD],
                                            rz[:, qc, :], None, ALU.mult)
        attnT = blk1.tile([P, LS, TOK], BF16, tag="attnT")
        for s in range(LS):
            pst = psum.tile([P, TC, P], BF16, tag="ps")
            for c in range(TC):
                nc.tensor.transpose(pst[:, c, :], attn[:, c, ts(s, P)],
                                    ident[:])
            nc.scalar.activation(attnT[:, s, :], pst[:], AF.Copy)
        if nxt is not None:
            with _prep_prio():
                prep_c(dn, nxt)

        # --- attn proj (gate pre-folded into pjw) + residual ---
        for c in range(TC):
            pspj = psum.tile([P, LAT], F32, tag="ps")
            for s in range(LS):
                nc.tensor.matmul(pspj[:], attnT[:, s, ts(c, P)], pjw[:, s, :],
                                 start=(s == 0), stop=(s == LS - 1))
            nc.vector.tensor_tensor(xc[:, c, :], xc[:, c, :], pspj[:],
                                    ALU.add)

        # --- LN2 + modulate + transpose ---
        hh2T = blk2x.tile([P, LS, TOK], BF16, tag="hh2T")
        _ln_modulate_transpose(nc, small, blk1, psum, ident, xc[:],
                               adaT[:, 3, :], adaT[:, 2, :], eps6, hh2T[:],
                               "l2")
        if nxt is not None:
            with _prep_prio():
                prep_d(dn, nxt)

        # --- MLP; gelu(x) ~= x * sigmoid(1.702 x) = x / (1 + exp(-1.702 x))
        #     (rel err vs exact erf-gelu ~1.4e-4 end to end; keeps the whole
        #      steady-state loop in the natural_log_exp table set) ---
        mlpT = blk2x.tile([P, MS, TOK], BF16, tag="mlpT")
        for mp in range(MS // 2):
            psf = psum.tile([P, 2, TOK], F32, tag="ps")
            for half in range(2):
                m = 2 * mp + half
                for s in range(LS):
                    nc.tensor.matmul(psf[:, half, :], f1w[:, s, ts(m, P)],
                                     hh2T[:, s, :], start=(s == 0),
                                     stop=(s == LS - 1))
            ge = blk1.tile([P, 2, TOK], BF16, tag="gelu_e")
            nc.scalar.activation(ge[:], psf[:], AF.Exp, scale=-1.702)
            nc.vector.tensor_scalar(ge[:], ge[:], 1.0, None, ALU.add)
            gr = blk1.tile([P, 2, TOK], F32, tag="gelu_r")
            nc.vector.reciprocal(gr[:], ge[:])
            nc.vector.tensor_tensor(mlpT[:, 2 * mp:2 * mp + 2, :], psf[:],
                                    gr[:], ALU.mult)
        if nxt is not None:
            with _prep_prio():
                prep_e(dn, nxt)
        for c in range(TC):
            psm2 = psum.tile([P, LAT], F32, tag="ps")
            for s in range(MS):
                nc.tensor.matmul(psm2[:], mlpT[:, s, ts(c, P)], f2w[:, s, :],
                                 start=(s == 0), stop=(s == MS - 1))
            nc.vector.tensor_tensor(xc[:, c, :], xc[:, c, :], psm2[:],
                                    ALU.add)

    st = prep_block(0)
    for d in range(depth):
        nxt = {} if d + 1 < depth else None
        main_block(d, st, d + 1, nxt)
        st = nxt

    # ---------------- final layer (borrows weight-pool slots) ----------------
    finada = wq.tile([P, LS, 2 * LAT], BF16, tag="we")
    nc.sync.dma_start(finada[:], finada_d)
    finw = wq.tile([P, LS, OUT_C], BF16, tag="qpv")
    nc.sync.dma_start(finw[:], finw_d)
    adaf_row = once.tile([1, 16 * P], BF16, tag="adaf_row")
    nc.vector.memset(adaf_row[:], 0.0)
    for n in range(2):
        psa = psrow.tile([1, LAT], F32, tag="ps_row")
        for s in range(LS):
            nc.tensor.matmul(psa[:], sT[:, s, :], finada[:, s, ts(n, LAT)],
                             start=(s == 0), stop=(s == LS - 1))
        nc.scalar.activation(adaf_row[:, ts(n, LAT)], psa[:], AF.Copy)
    adaf_dr = dram.tile([1, 16 * P], BF16, tag="adaf_dr")
    nc.sync.dma_start(adaf_dr[:], adaf_row[:])
    adaTf16 = once.tile([P, 16], BF16, tag="adaTf16")
    nc.sync.dma_start_transpose(
        adaTf16[:], adaf_dr[:].rearrange("o (r c) -> (o r) c", r=16, c=P))
    adaTf = once.tile([P, 2, LS], F32, tag="adaTf")
    nc.vector.tensor_copy(
        adaTf[:], adaTf16[:, 0:8].rearrange("p (v s) -> p v s", v=2))
    nc.vector.tensor_scalar(adaTf[:, 1, :], adaTf[:, 1, :], 1.0, None,
                            ALU.add)
    hhfT = blk1.tile([P, LS, TOK], BF16, tag="hh1T")
    _ln_modulate_transpose(nc, small, blk1, psum, ident, xc[:],
                           adaTf[:, 1, :], adaTf[:, 0, :], eps6, hhfT[:],
                           "l1")
    outsb = blk1.tile([P, TC, OUT_C], F32, tag="attnT")
    for mc in range(TC):
        pso = psum.tile([P, OUT_C], F32, tag="ps")
        for s in range(LS):
            nc.tensor.matmul(pso[:], finw[:, s, ts(mc, P)], hhfT[:, s, :],
                             start=(s == 0), stop=(s == LS - 1))
        nc.scalar.activation(outsb[:, mc, :], pso[:], AF.Copy)
    nc.sync.dma_start(out_t, outsb[:])


# ---------------------------------------------------------------------------
# host side
# ---------------------------------------------------------------------------

def _to_bf16(a):
    return np.asarray(a, dtype=np.float32).astype(ml_dtypes.bfloat16)


def _stage_wT(w, S, N):
    """w: [..., N_out, K] -> W^T staged [..., 128, S, N_out] bf16."""
    wt = np.ascontiguousarray(np.swapaxes(np.asarray(w, np.float32), -1, -2))
    shp = wt.shape
    K, M = shp[-2], shp[-1]
    assert K == S * P and M == N, (shp, S, N)
    wt = wt.reshape(shp[:-2] + (S, P, M))
    wt = np.swapaxes(wt, -3, -2)  # [..., P, S, M]
    return _to_bf16(np.ascontiguousarray(wt))


_CACHE = {}


def _get_program():
    if DEPTH not in _CACHE:
        _CACHE[DEPTH] = build_program(DEPTH)
    return _CACHE[DEPTH]


def prepare_in_maps(inputs, depth=DEPTH):
    f32 = np.float32
    x = np.asarray(inputs["x"], f32)
    t = np.asarray(inputs["t"], f32)
    coords = np.asarray(inputs["coords"], f32)

    qkv = np.asarray(inputs["blk_qkv_w"], f32)[:depth]
    qk = qkv[:, :2 * LAT, :].copy()
    qk[:, :LAT, :] *= HD ** -0.5
    vw = qkv[:, 2 * LAT:, :]

    common = {
        "ident": _to_bf16(np.eye(P)),
        "div_bc": np.ascontiguousarray(np.broadcast_to(
            np.exp(np.arange(0, LAT, 2, dtype=f32)
                   * (-math.log(10000.0) / LAT)),
            (P, TC, 256)).astype(f32)),
        "projw": _stage_wT(inputs["proj_w"], 1, LAT),
        "te1": _stage_wT(inputs["te_w1"], 2, LAT),
        "te2": _stage_wT(inputs["te_w2"], LS, LAT),
        # ada rows reordered to [sh_a|sc_a|sh_m|sc_m|g_a|g_m] so the two
        # gates broadcast with one DMA and the shift/scale gather is 4-wide
        "we_w": np.concatenate([
            _stage_wT(np.asarray(inputs["blk_ada_w"], f32)[:depth]
                      .reshape(depth, 6, LAT, LAT)[:, [0, 1, 3, 4, 2, 5]]
                      .reshape(depth, 6 * LAT, LAT), LS, 6 * LAT),
            _stage_wT(np.asarray(inputs["blk_pe1_w"], f32)[:depth], LS, LAT),
            _stage_wT(np.asarray(inputs["blk_pe2_w"], f32)[:depth], LS, LAT),
        ], axis=3),
        "qpv_w": np.concatenate([
            _stage_wT(qk, LS, 2 * LAT),
            _stage_wT(vw, LS, LAT),
            _stage_wT(np.asarray(inputs["blk_proj_w"], f32)[:depth],
                      LS, LAT),
        ], axis=3),
        "f1_w": _stage_wT(np.asarray(inputs["blk_fc1_w"], f32)[:depth],
                          LS, MLP_H),
        "f2_w": _stage_wT(np.asarray(inputs["blk_fc2_w"], f32)[:depth],
                          MS, LAT),
        "finada": _stage_wT(inputs["fin_ada_w"], LS, 2 * LAT),
        "finw": _stage_wT(inputs["fin_w"], LS, OUT_C),
    }
    in_maps = []
    for b in range(B):
        m = dict(common)
        m["x_t"] = np.ascontiguousarray(
            x[b].T.reshape(TC, P, HID).swapaxes(0, 1))
        m["coords_t"] = np.ascontiguousarray(
            coords[b].T.reshape(TC, P, 3).swapaxes(0, 1))
        cv = np.zeros((P, 8), f32)
        cv[:, 0] = -math.pi
        cv[:, 1] = 1e-5
        cv[:, 2] = 1e-6
        cv[:, 3] = np.exp(-math.log(10000.0)
                          * np.arange(FREQ // 2, dtype=f32) / (FREQ // 2))
        cv[:, 4] = t[b]
        cv[:, 5] = (np.arange(P) // 64 == 0)
        cv[:, 6] = (np.arange(P) // 64 == 1)
        m["cvec"] = cv
        s2t = np.zeros((2, P), f32)
        s2t[0, 0:64] = 1.0
        s2t[1, 64:P] = 1.0
        m["sel2T"] = s2t
        in_maps.append(m)
    return in_maps


def run_spmd(inputs, **kw):
    nc = _get_program()
    in_maps = prepare_in_maps(inputs, DEPTH)
    res = run_bass_kernel_spmd(nc, in_maps, core_ids=list(range(B)), **kw)
    outs = []
    for b in range(B):
        o = np.asarray(res.results[b]["out_t"], np.float32)  # [P, TC, OUT]
        outs.append(o.transpose(1, 0, 2).reshape(OUT_C, TOK))
    return np.stack(outs, axis=0), res


def kernel(**inputs):
    out, _ = run_spmd(inputs)
    return out

